# revision 3
# baseline (speedup 1.0000x reference)
"""Sliding-window causal GQA self-attention (B=2, T=2048, 16 q-heads, 4 kv-heads,
head_dim=128, window=1024) on 8 trn2 NeuronCores.

Sharding: core = (batch b, kv-group g) -> 4 query heads + 1 kv head, full T.
Wo is row-parallel; each core emits a [T, 2048] bf16 partial that the host
upcasts and sums per batch (the unshard step for the row-parallel layout).

Precision plan (gate is rel-err < 2e-2 vs f32 reference; this lands ~4e-3):
  - QKV projections run as fp8e4m3 DoubleRow matmuls (2 contraction rows per
    partition, 0.5 PE cycles/row = 4x f32 rate) in an error-compensated
    3-chain form: x = xh + xl (host hi/lo split), W = (Wh + Wl)/64 (host
    split, x64 pre-scale keeps W out of the fp8 subnormal range), computing
    xh@Wh + xl@Wh + xh@Wl (the xl@Wl term is ~1e-4 relative and dropped).
    The 64x output scale cancels inside RMS-norm for q/k (bias = eps*64^2)
    and is folded into Wo on the host for the v path.
  - Everything else is bf16 (1 PE cycle/row, 2x DVE mode, half DMA): rope
    tables, masks, pt=exp(S), V, y, Wo, output. f32 only in PSUM, RMS-norm
    scales, softmax reciprocals, and the ve gate.

Device dataflow:
  phase 1: qT/kT/vT projections (3-chain fp8 DR), RoPE (half-swap DMA +
           [c;c], [s;-s] tables), RMS-norm via squared-input all-ones-matmul
           replicated sum; raw v^T parked in SBUF. ACT runs Sqrt only.
  phase 1b: gate sigmoid via Exp (bf16 x-stripe matmul), v = v_raw + gated ve,
           PE-transpose of v^T into natural V. One Exp act-table load that
           phase 2 reuses (act-table thrash was ~27 loads x 1.3us).
  phase 2: S^T = K^T.T @ Q^T per 128-key block x 256-query super; ACT exp
           (scale fused) -> bf16; 0/1 triangle masks for window edges; PV and
           all-ones rowsum accumulated in PSUM; normalize on evacuation
           (y^T overwrites the dead q^T slice).
  phase 3: out[t, o] = sum_h yT_h^T @ Wo_h, Wo streamed per 512-col slice.
"""

import numpy as np

B, T, E = 2, 2048, 2048
NH, NKV, HD = 16, 4, 128
GATE_C = 32
WIN = 1024
EPS = 1e-6
NE = E // 128          # 16 contraction chunks
NE2 = NE // 2          # 8 fp8 DoubleRow pair-chunks
TC = 256               # phase-1 token chunk (= q-super width)
NTC = T // TC          # 8
NKB = T // 128         # 16 key blocks
SCALE = 1.0 / np.sqrt(HD)
WSC = 64.0             # host pre-scale on Wq/Wk/Wv for fp8 range

_CACHE = {}


def _build_program():
    import concourse.bacc as bacc
    import concourse.mybir as mybir
    import concourse.tile as tile

    F32, BF16, FP8 = mybir.dt.float32, mybir.dt.bfloat16, mybir.dt.float8e4
    AF = mybir.ActivationFunctionType
    OP = mybir.AluOpType
    DR = mybir.MatmulPerfMode.DoubleRow

    nc = bacc.Bacc("TRN2", target_bir_lowering=False, debug=False, num_devices=8)

    xh8 = nc.dram_tensor("xh8", [E, T], FP8, kind="ExternalInput")
    xl8 = nc.dram_tensor("xl8", [E, T], FP8, kind="ExternalInput")
    xg = nc.dram_tensor("xg", [GATE_C, T], BF16, kind="ExternalInput")
    veT = nc.dram_tensor("veT", [HD, T], BF16, kind="ExternalInput")
    crep = nc.dram_tensor("crep", [128, T], BF16, kind="ExternalInput")
    ssgn = nc.dram_tensor("ssgn", [128, T], BF16, kind="ExternalInput")
    wqh = nc.dram_tensor("wqh", [E, 512], FP8, kind="ExternalInput")
    wql = nc.dram_tensor("wql", [E, 512], FP8, kind="ExternalInput")
    wkh = nc.dram_tensor("wkh", [E, HD], FP8, kind="ExternalInput")
    wkl = nc.dram_tensor("wkl", [E, HD], FP8, kind="ExternalInput")
    wvh = nc.dram_tensor("wvh", [E, HD], FP8, kind="ExternalInput")
    wvl = nc.dram_tensor("wvl", [E, HD], FP8, kind="ExternalInput")
    wg = nc.dram_tensor("wg", [GATE_C, 128], BF16, kind="ExternalInput")
    wo = nc.dram_tensor("wo", [512, E], BF16, kind="ExternalInput")
    m_in = nc.dram_tensor("m_in", [4, 128, 512], BF16, kind="ExternalInput")
    mn_in = nc.dram_tensor("mn_in", [128, 256], BF16, kind="ExternalInput")
    ones_in = nc.dram_tensor("ones_in", [128, 128], BF16, kind="ExternalInput")
    eye_in = nc.dram_tensor("eye_in", [128, 128], BF16, kind="ExternalInput")
    out = nc.dram_tensor("out", [T, E], BF16, kind="ExternalOutput")

    xh_r = xh8.rearrange("(e k) t -> k e t", k=128)
    xl_r = xl8.rearrange("(e k) t -> k e t", k=128)
    wqh_r = wqh.rearrange("(e k) d -> k e d", k=128)
    wql_r = wql.rearrange("(e k) d -> k e d", k=128)
    wkh_r = wkh.rearrange("(e k) d -> k e d", k=128)
    wkl_r = wkl.rearrange("(e k) d -> k e d", k=128)
    wvh_r = wvh.rearrange("(e k) d -> k e d", k=128)
    wvl_r = wvl.rearrange("(e k) d -> k e d", k=128)

    with tile.TileContext(nc) as tc:
        from contextlib import ExitStack
        with ExitStack() as ctx:
            cst = ctx.enter_context(tc.tile_pool(name="cst", bufs=1))
            wts = ctx.enter_context(tc.tile_pool(name="wts", bufs=1))
            xtp = ctx.enter_context(tc.tile_pool(name="xtp", bufs=2))
            res = ctx.enter_context(tc.tile_pool(name="res", bufs=1))
            qrp = ctx.enter_context(tc.tile_pool(name="qrp", bufs=5))
            wk1 = ctx.enter_context(tc.tile_pool(name="wk1", bufs=4))
            wk2 = ctx.enter_context(tc.tile_pool(name="wk2", bufs=2))
            ptp = ctx.enter_context(tc.tile_pool(name="ptp", bufs=4))
            wop = ctx.enter_context(tc.tile_pool(name="wop", bufs=2))
            stg = ctx.enter_context(tc.tile_pool(name="stg", bufs=4))
            p_q = ctx.enter_context(tc.tile_pool(name="p_q", bufs=2, space="PSUM"))
            p_sm = ctx.enter_context(tc.tile_pool(name="p_sm", bufs=1, space="PSUM"))
            p_s = ctx.enter_context(tc.tile_pool(name="p_s", bufs=3, space="PSUM"))
            p_or = ctx.enter_context(tc.tile_pool(name="p_or", bufs=2, space="PSUM"))

            # ---- small constants ----
            masks_sb = cst.tile([128, 4, 512], BF16, tag="masks")
            masksn_sb = cst.tile([128, 256], BF16, tag="masksn")
            ones_sb = cst.tile([128, 128], BF16, tag="ones")
            eye_sb = cst.tile([128, 128], BF16, tag="eye")
            eps_sb = cst.tile([128, 1], F32, tag="eps")
            nc.sync.dma_start(out=masks_sb, in_=m_in.rearrange("m p f -> p m f"))
            nc.sync.dma_start(out=masksn_sb, in_=mn_in[:])
            nc.sync.dma_start(out=ones_sb, in_=ones_in[:])
            nc.sync.dma_start(out=eye_sb, in_=eye_in[:])
            nc.vector.memset(eps_sb, EPS * WSC * WSC)

            wg_sb = wts.tile([GATE_C, 128], BF16, tag="wg")
            nc.sync.dma_start(out=wg_sb, in_=wg[:])

            # ---- chunk-0 stream DMAs FIRST so compute starts early ----
            xht0 = xtp.tile([128, NE, TC], FP8, tag="xht")
            xlt0 = xtp.tile([128, NE, TC], FP8, tag="xlt")
            for e4 in range(4):
                sl = slice(e4 * 4, (e4 + 1) * 4)
                nc.sync.dma_start(out=xht0[:, sl, :], in_=xh_r[:, sl, 0:TC])
                nc.sync.dma_start(out=xlt0[:, sl, :], in_=xl_r[:, sl, 0:TC])

            # ---- persistent streams: rope tables, gate stripe, ve ----
            crep_sb = wts.tile([128, T], BF16, tag="crep")
            ssgn_sb = wts.tile([128, T], BF16, tag="ssgn")
            xg_sb = wts.tile([GATE_C, T], BF16, tag="xg")
            vef_sb = wts.tile([HD, T], BF16, tag="vef")
            nc.sync.dma_start(out=crep_sb, in_=crep[:])
            nc.sync.dma_start(out=ssgn_sb, in_=ssgn[:])
            nc.sync.dma_start(out=xg_sb, in_=xg[:])
            nc.sync.dma_start(out=vef_sb, in_=veT[:])

            # ---- weights, split by e-chunk groups (interleaved queues) ----
            wqh_sb = wts.tile([128, NE, 512], FP8, tag="wqh")
            wql_sb = wts.tile([128, NE, 512], FP8, tag="wql")
            wkh_sb = wts.tile([128, NE, HD], FP8, tag="wkh")
            wkl_sb = wts.tile([128, NE, HD], FP8, tag="wkl")
            wvh_sb = wts.tile([128, NE, HD], FP8, tag="wvh")
            wvl_sb = wts.tile([128, NE, HD], FP8, tag="wvl")
            for e4 in range(4):
                sl = slice(e4 * 4, (e4 + 1) * 4)
                nc.sync.dma_start(out=wqh_sb[:, sl, :], in_=wqh_r[:, sl, :])
                nc.sync.dma_start(out=wql_sb[:, sl, :], in_=wql_r[:, sl, :])
                nc.sync.dma_start(out=wkh_sb[:, sl, :], in_=wkh_r[:, sl, :])
                nc.sync.dma_start(out=wkl_sb[:, sl, :], in_=wkl_r[:, sl, :])
                nc.sync.dma_start(out=wvh_sb[:, sl, :], in_=wvh_r[:, sl, :])
                nc.sync.dma_start(out=wvl_sb[:, sl, :], in_=wvl_r[:, sl, :])

            # ---- persistent results (yT overwrites qT slices in phase 2) ----
            qyT_sb = res.tile([128, 4, T], BF16, tag="qyT")
            kT_sb = res.tile([128, T], BF16, tag="kT")
            vn_sb = res.tile([128, NKB, HD], BF16, tag="vn")
            vraw_sb = res.tile([128, NTC, TC], BF16, tag="vraw")

            def proj_chains(ps, wh_sb, wl_sb, xh_t, xl_t, dsl):
                """3-chain fp8 DoubleRow projection into PSUM ps."""
                n = 3 * NE2
                i = 0
                for w_sb, x_t in ((wh_sb, xh_t), (wl_sb, xh_t), (wh_sb, xl_t)):
                    for e2 in range(NE2):
                        nc.tensor.matmul(
                            ps, w_sb[:, 2 * e2:2 * e2 + 2, dsl], x_t[:, 2 * e2:2 * e2 + 2, :],
                            start=(i == 0), stop=(i == n - 1), perf_mode=DR)
                        i += 1

            def emit_attn(hp, qs):
                h2 = slice(2 * hp, 2 * hp + 2)
                q0 = qs * TC
                kb0 = max(0, 2 * qs - 8)
                kb1 = 2 * qs + 2
                o_ps = p_or.tile([128, 512], F32, tag="or")
                r_ps = p_or.tile([128, 512], F32, tag="or")
                for kb in range(kb0, kb1):
                    if kb == 2 * qs + 1:
                        # diag end: only q-high halves live; contributes to the
                        # (h, q-high) sub-columns, start=False (never first)
                        rhs_hi = qyT_sb[:, h2, q0 + 128:q0 + 256]
                        s_n = p_s.tile([128, 256], F32, tag="s")
                        nc.tensor.matmul(s_n, kT_sb[:, kb * 128:(kb + 1) * 128],
                                         rhs_hi, start=True, stop=True)
                        pt_n = ptp.tile([128, 256], BF16, tag="pt")
                        nc.scalar.activation(pt_n, s_n, AF.Exp, scale=float(SCALE))
                        nc.vector.tensor_tensor(pt_n, pt_n, masksn_sb, OP.mult)
                        o_v = o_ps.rearrange("p (h q) -> p h q", h=2)
                        r_v = r_ps.rearrange("p (h q) -> p h q", h=2)
                        nc.tensor.matmul(o_v[:, :, 128:256], vn_sb[:, kb, :], pt_n,
                                         start=False, stop=True, skip_group_check=True)
                        nc.tensor.matmul(r_v[:, :, 128:256], ones_sb, pt_n,
                                         start=False, stop=True, skip_group_check=True)
                        continue
                    s_ps = p_s.tile([128, 512], F32, tag="s")
                    nc.tensor.matmul(s_ps,
                                     kT_sb[:, kb * 128:(kb + 1) * 128],
                                     qyT_sb[:, h2, q0:q0 + TC],
                                     start=True, stop=True)
                    pt = ptp.tile([128, 512], BF16, tag="pt")
                    nc.scalar.activation(pt, s_ps, AF.Exp, scale=float(SCALE))
                    mi = None
                    if kb == 2 * qs:
                        mi = 0
                    elif qs >= 4 and kb == kb0:
                        mi = 2
                    elif qs >= 4 and kb == kb0 + 1:
                        mi = 3
                    if mi is not None:
                        nc.vector.tensor_tensor(pt, pt, masks_sb[:, mi, :], OP.mult)
                    nc.tensor.matmul(o_ps, vn_sb[:, kb, :], pt,
                                     start=(kb == kb0), stop=False, skip_group_check=True)
                    nc.tensor.matmul(r_ps, ones_sb, pt,
                                     start=(kb == kb0), stop=False, skip_group_check=True)
                rr = wk2.tile([128, 512], F32, tag="rr")
                nc.vector.reciprocal(rr, r_ps)
                nc.vector.tensor_mul(qyT_sb[:, h2, q0:q0 + TC], o_ps, rr)

            # ================= phase 1: projections + rms/rope (Sqrt only) ====
            for tcix in range(NTC):
                ts = tcix * TC
                if tcix == 0:
                    xh_t, xl_t = xht0, xlt0
                else:
                    xh_t = xtp.tile([128, NE, TC], FP8, tag="xht")
                    xl_t = xtp.tile([128, NE, TC], FP8, tag="xlt")
                    nc.sync.dma_start(out=xh_t, in_=xh_r[:, :, ts:ts + TC])
                    nc.sync.dma_start(out=xl_t, in_=xl_r[:, :, ts:ts + TC])
                c_sl = crep_sb[:, ts:ts + TC]
                s_sl = ssgn_sb[:, ts:ts + TC]

                # projections + rms + rope; sumsq paired per 2 srcs so each
                # Sqrt covers two sources (fewer act-table switches)
                srcs = [("q", 0), ("q", 1), ("q", 2), ("q", 3), ("k", 0)]
                chunk_qraws = []
                ss_pair = None
                rr_pair = None
                for i, (kind, h) in enumerate(srcs):
                    ps = p_q.tile([128, TC], F32, tag="q")
                    if kind == "q":
                        proj_chains(ps, wqh_sb, wql_sb, xh_t, xl_t,
                                    slice(h * 128, (h + 1) * 128))
                    else:
                        proj_chains(ps, wkh_sb, wkl_sb, xh_t, xl_t, slice(0, HD))
                    qraw = qrp.tile([128, TC], BF16, tag="qraw")
                    nc.vector.tensor_copy(qraw, ps)
                    chunk_qraws.append(qraw)
                    sq = wk1.tile([128, TC], BF16, tag="sq")
                    nc.vector.tensor_mul(sq, qraw, qraw)
                    half = i % 2
                    if half == 0:
                        ss_pair = p_sm.tile([128, 512], F32, tag="small")
                        rr_pair = wk2.tile([128, 512], F32, tag="rrms")
                    nc.tensor.matmul(ss_pair[:, half * TC:(half + 1) * TC],
                                     ones_sb, sq, start=True, stop=True)
                    if half == 1 or i == 4:
                        wd = 512 if half == 1 else 256
                        nc.scalar.activation(rr_pair[:, 0:wd], ss_pair[:, 0:wd],
                                             AF.Sqrt, bias=eps_sb, scale=1.0 / HD)
                        nc.vector.reciprocal(rr_pair[:, 0:wd], rr_pair[:, 0:wd])
                        done = [i - 1, i] if half == 1 else [i]
                        for ii in done:
                            kind2, h2 = srcs[ii]
                            qraw2 = chunk_qraws[ii]
                            rrms = rr_pair[:, (ii % 2) * TC:(ii % 2 + 1) * TC]
                            qsw = wk1.tile([128, TC], BF16, tag="qsw")
                            nc.sync.dma_start(out=qsw[0:64, :], in_=qraw2[64:128, :])
                            nc.sync.dma_start(out=qsw[64:128, :], in_=qraw2[0:64, :])
                            tA = wk1.tile([128, TC], BF16, tag="tA")
                            tB = wk1.tile([128, TC], BF16, tag="tB")
                            nc.vector.tensor_mul(tA, qraw2, c_sl)
                            nc.gpsimd.tensor_tensor(tB, qsw, s_sl, OP.mult)
                            nc.vector.tensor_add(tA, tA, tB)
                            dest = (qyT_sb[:, h2, ts:ts + TC] if kind2 == "q"
                                    else kT_sb[:, ts:ts + TC])
                            nc.vector.tensor_mul(dest, tA, rrms)

                # v: projection only; gated ve mixing happens in phase 1b
                ps_v = p_q.tile([128, TC], F32, tag="q")
                proj_chains(ps_v, wvh_sb, wvl_sb, xh_t, xl_t, slice(0, HD))
                nc.vector.tensor_copy(vraw_sb[:, tcix, :], ps_v)

            # ================= phase 1b: gate + v mix + transpose (Exp table) ==
            for tcix in range(NTC):
                ts = tcix * TC
                # gate via exp: g = 1/(1+exp(-u)); the 2x (and v's 64x) folds
                # into the STT scalar
                g_ps = p_sm.tile([128, TC], F32, tag="small")
                nc.tensor.matmul(g_ps, wg_sb, xg_sb[:, ts:ts + TC], start=True, stop=True)
                g_rep = wk2.tile([128, TC], F32, tag="grep")
                nc.scalar.activation(g_rep, g_ps, AF.Exp, scale=-1.0)
                nc.vector.tensor_scalar_add(g_rep, g_rep, 1.0)
                nc.vector.reciprocal(g_rep, g_rep)
                tv = wk1.tile([128, TC], BF16, tag="tA")
                nc.gpsimd.tensor_tensor(tv, vef_sb[:, ts:ts + TC], g_rep, OP.mult)
                vt = wk1.tile([128, TC], BF16, tag="tB")
                nc.vector.scalar_tensor_tensor(vt, tv, 2.0 * WSC, vraw_sb[:, tcix, :],
                                               OP.mult, OP.add)
                for tb in range(TC // 128):
                    tp_ps = p_sm.tile([128, 128], BF16, tag="small")
                    nc.tensor.transpose(tp_ps, vt[:, tb * 128:(tb + 1) * 128], eye_sb)
                    nc.vector.tensor_copy(vn_sb[:, tcix * 2 + tb, :], tp_ps)

            # ================= phase 2: windowed attention (head-paired) =======
            for qs in range(NTC):
                for hp in range(2):
                    emit_attn(hp, qs)

            # ================= phase 3: out = y @ Wo (row-parallel partial) ====
            for os_ in range(4):
                wo_sl = wop.tile([128, 4, 512], BF16, tag="wo")
                nc.sync.dma_start(
                    out=wo_sl,
                    in_=wo.rearrange("(h d) o -> d h o", d=128)[:, :, os_ * 512:(os_ + 1) * 512],
                )
                for tt in range(T // 128):
                    pool3, tag3 = (p_s, "s") if tt % 2 == 0 else (p_or, "or")
                    po = pool3.tile([128, 512], F32, tag=tag3)
                    for h in range(4):
                        nc.tensor.matmul(po, qyT_sb[:, h, tt * 128:(tt + 1) * 128],
                                         wo_sl[:, h, :], start=(h == 0), stop=(h == 3))
                    stage = stg.tile([128, 512], BF16, tag="stage")
                    if tt % 2 == 0:
                        nc.vector.tensor_copy(stage, po)
                    else:
                        nc.scalar.copy(stage, po)
                    nc.sync.dma_start(
                        out=out[tt * 128:(tt + 1) * 128, os_ * 512:(os_ + 1) * 512],
                        in_=stage)

    nc.compile()
    return nc


def _masks():
    jj = np.arange(128)[:, None]
    ii = np.arange(128)[None, :]
    tri_d = (jj <= ii).astype(np.float32)   # diag block: keep j <= i
    tri_f = (jj >= ii).astype(np.float32)   # far block: keep j >= i - WIN
    one = np.ones((128, 128), np.float32)
    zero = np.zeros((128, 128), np.float32)
    m0 = np.concatenate([tri_d, one], 1)
    m1 = np.concatenate([zero, tri_d], 1)
    m2 = np.concatenate([tri_f, zero], 1)
    m3 = np.concatenate([one, tri_f], 1)
    base = np.ascontiguousarray(np.tile(np.stack([m0, m1, m2, m3]), (1, 1, 2)))
    mn = np.ascontiguousarray(np.concatenate([tri_d, tri_d], 1))
    return base, mn


def _hilo(a, scale=1.0):
    import ml_dtypes
    F8 = ml_dtypes.float8_e4m3
    s = (a * scale).astype(np.float32)
    h = s.astype(F8)
    l = (s - h.astype(np.float32)).astype(F8)
    return np.ascontiguousarray(h), np.ascontiguousarray(l)


def kernel(**inputs):
    import ml_dtypes
    from concourse.bass_utils import run_bass_kernel_spmd

    BF = ml_dtypes.bfloat16

    if "nc" not in _CACHE:
        _CACHE["nc"] = _build_program()
    nc = _CACHE["nc"]

    x = np.asarray(inputs["x"], np.float32)
    ve = np.asarray(inputs["ve"], np.float32)
    cos = np.asarray(inputs["cos"], np.float32)
    sin = np.asarray(inputs["sin"], np.float32)
    Wq = np.asarray(inputs["Wq"], np.float32)
    Wk = np.asarray(inputs["Wk"], np.float32)
    Wv = np.asarray(inputs["Wv"], np.float32)
    Wo = np.asarray(inputs["Wo"], np.float32)
    Wg = np.asarray(inputs["Wg"], np.float32)

    crep = np.ascontiguousarray(np.concatenate([cos.T, cos.T], 0)).astype(BF)
    ssgn = np.ascontiguousarray(np.concatenate([sin.T, -sin.T], 0)).astype(BF)
    masks, masksn = _masks()
    masks = masks.astype(BF)
    masksn = masksn.astype(BF)
    ones128 = np.ones((128, 128), BF)
    eye128 = np.eye(128, dtype=BF)

    in_maps = []
    for c in range(8):
        b, g = divmod(c, 4)
        xT = np.ascontiguousarray(x[b].T)
        xh, xl = _hilo(xT)
        wq_h, wq_l = _hilo(Wq[:, g * 512:(g + 1) * 512], WSC)
        wk_h, wk_l = _hilo(Wk[:, g * HD:(g + 1) * HD], WSC)
        wv_h, wv_l = _hilo(Wv[:, g * HD:(g + 1) * HD], WSC)
        in_maps.append({
            "xh8": xh,
            "xl8": xl,
            "xg": np.ascontiguousarray(xT[:GATE_C]).astype(BF),
            "veT": np.ascontiguousarray(ve[b, :, g * HD:(g + 1) * HD].T).astype(BF),
            "crep": crep,
            "ssgn": ssgn,
            "wqh": wq_h, "wql": wq_l,
            "wkh": wk_h, "wkl": wk_l,
            "wvh": wv_h, "wvl": wv_l,
            "wg": np.ascontiguousarray(np.repeat(Wg[:, g:g + 1], 128, 1)).astype(BF),
            "wo": np.ascontiguousarray(Wo[g * 512:(g + 1) * 512, :] / WSC).astype(BF),
            "m_in": masks,
            "mn_in": masksn,
            "ones_in": ones128,
            "eye_in": eye128,
        })

    res = run_bass_kernel_spmd(nc, in_maps, core_ids=list(range(8)))
    parts = [np.asarray(res.results[c]["out"]).astype(np.float32) for c in range(8)]
    out = np.stack([parts[0] + parts[1] + parts[2] + parts[3],
                    parts[4] + parts[5] + parts[6] + parts[7]])
    return out.astype(np.float32)


# revision 9
# speedup vs baseline: 1.0887x; 1.0887x over previous
"""Sliding-window causal GQA self-attention (B=2, T=2048, 16 q-heads, 4 kv-heads,
head_dim=128, window=1024) on 8 trn2 NeuronCores.

Sharding: core = (batch b, kv-group g) -> 4 query heads + 1 kv head, full T.
Wo is row-parallel; each core emits a [T, 2048] bf16 partial that the host
upcasts and sums per batch (the unshard step for the row-parallel layout).

Precision plan (gate is rel-err < 2e-2 vs f32 reference; this lands ~4e-3):
  - QKV projections run as fp8e4m3 DoubleRow matmuls (2 contraction rows per
    partition, 0.5 PE cycles/row = 4x f32 rate) in an error-compensated
    3-chain form: x = xh + xl (host hi/lo split), W = (Wh + Wl)/64 (host
    split, x64 pre-scale keeps W out of the fp8 subnormal range), computing
    xh@Wh + xl@Wh + xh@Wl (the xl@Wl term is ~1e-4 relative and dropped).
    The 64x output scale cancels inside RMS-norm for q/k (bias = eps*64^2)
    and is folded into Wo on the host for the v path.
  - Everything else is bf16 (1 PE cycle/row, 2x DVE mode, half DMA): rope
    tables, masks, pt=exp(S), V, y, Wo, output. f32 only in PSUM, RMS-norm
    scales, softmax reciprocals, and the ve gate.

Device dataflow:
  phase 1: qT/kT/vT projections (3-chain fp8 DR), RoPE (half-swap DMA +
           [c;c], [s;-s] tables), RMS-norm via squared-input all-ones-matmul
           replicated sum; raw v^T parked in SBUF. ACT runs Sqrt only.
  phase 1b: gate sigmoid via Exp (bf16 x-stripe matmul), v = v_raw + gated ve,
           PE-transpose of v^T into natural V. One Exp act-table load that
           phase 2 reuses (act-table thrash was ~27 loads x 1.3us).
  phase 2: S^T = K^T.T @ Q^T per 128-key block x 256-query super; ACT exp
           (scale fused) -> bf16; 0/1 triangle masks for window edges; PV and
           all-ones rowsum accumulated in PSUM; normalize on evacuation
           (y^T overwrites the dead q^T slice).
  phase 3: out[t, o] = sum_h yT_h^T @ Wo_h, Wo streamed per 512-col slice.
"""

import numpy as np

B, T, E = 2, 2048, 2048
NH, NKV, HD = 16, 4, 128
GATE_C = 32
WIN = 1024
EPS = 1e-6
NE = E // 128          # 16 contraction chunks
NE2 = NE // 2          # 8 fp8 DoubleRow pair-chunks
TC = 256               # phase-1 token chunk (= q-super width)
NTC = T // TC          # 8
NKB = T // 128         # 16 key blocks
SCALE = 1.0 / np.sqrt(HD)
WSC = 64.0             # host pre-scale on Wq/Wk/Wv for fp8 range

_CACHE = {}


def _build_program():
    import concourse.bacc as bacc
    import concourse.mybir as mybir
    import concourse.tile as tile

    F32, BF16, FP8 = mybir.dt.float32, mybir.dt.bfloat16, mybir.dt.float8e4
    AF = mybir.ActivationFunctionType
    OP = mybir.AluOpType
    DR = mybir.MatmulPerfMode.DoubleRow

    nc = bacc.Bacc("TRN2", target_bir_lowering=False, debug=False, num_devices=8)

    # x and weights are host-packed chunk-major/partition-major so every DMA
    # reads >=2KB contiguous per partition (short runs pay 2x DMA latency)
    xh8 = nc.dram_tensor("xh8", [NTC, 128, NE, TC], FP8, kind="ExternalInput")
    xl8 = nc.dram_tensor("xl8", [NTC, 128, NE, TC], FP8, kind="ExternalInput")
    xg = nc.dram_tensor("xg", [GATE_C, T], BF16, kind="ExternalInput")
    veT = nc.dram_tensor("veT", [HD, T], BF16, kind="ExternalInput")
    crep = nc.dram_tensor("crep", [128, T], BF16, kind="ExternalInput")
    ssgn = nc.dram_tensor("ssgn", [128, T], BF16, kind="ExternalInput")
    wqh = nc.dram_tensor("wqh", [128, NE, 512], FP8, kind="ExternalInput")
    wql = nc.dram_tensor("wql", [128, NE, 512], FP8, kind="ExternalInput")
    wkh = nc.dram_tensor("wkh", [128, NE, HD], FP8, kind="ExternalInput")
    wkl = nc.dram_tensor("wkl", [128, NE, HD], FP8, kind="ExternalInput")
    wvh = nc.dram_tensor("wvh", [128, NE, HD], FP8, kind="ExternalInput")
    wvl = nc.dram_tensor("wvl", [128, NE, HD], FP8, kind="ExternalInput")
    wg = nc.dram_tensor("wg", [GATE_C, 128], BF16, kind="ExternalInput")
    wo = nc.dram_tensor("wo", [512, E], BF16, kind="ExternalInput")
    m_in = nc.dram_tensor("m_in", [4, 128, 512], BF16, kind="ExternalInput")
    mn_in = nc.dram_tensor("mn_in", [128, 256], BF16, kind="ExternalInput")
    ones_in = nc.dram_tensor("ones_in", [128, 128], BF16, kind="ExternalInput")
    eye_in = nc.dram_tensor("eye_in", [128, 128], BF16, kind="ExternalInput")
    out = nc.dram_tensor("out", [T, E], BF16, kind="ExternalOutput")

    with tile.TileContext(nc) as tc:
        from contextlib import ExitStack
        with ExitStack() as ctx:
            cst = ctx.enter_context(tc.tile_pool(name="cst", bufs=1))
            wts = ctx.enter_context(tc.tile_pool(name="wts", bufs=1))
            xtp = ctx.enter_context(tc.tile_pool(name="xtp", bufs=2))
            res = ctx.enter_context(tc.tile_pool(name="res", bufs=1))
            qrp = ctx.enter_context(tc.tile_pool(name="qrp", bufs=5))
            wk1 = ctx.enter_context(tc.tile_pool(name="wk1", bufs=4))
            wk2 = ctx.enter_context(tc.tile_pool(name="wk2", bufs=2))
            ptp = ctx.enter_context(tc.tile_pool(name="ptp", bufs=4))
            wop = ctx.enter_context(tc.tile_pool(name="wop", bufs=2))
            stg = ctx.enter_context(tc.tile_pool(name="stg", bufs=4))
            p_q = ctx.enter_context(tc.tile_pool(name="p_q", bufs=2, space="PSUM"))
            p_sm = ctx.enter_context(tc.tile_pool(name="p_sm", bufs=1, space="PSUM"))
            p_s = ctx.enter_context(tc.tile_pool(name="p_s", bufs=3, space="PSUM"))
            p_or = ctx.enter_context(tc.tile_pool(name="p_or", bufs=2, space="PSUM"))

            # ---- chunk-0 x + first weights FIRST so compute starts early ----
            xht0 = xtp.tile([128, NE, TC], FP8, tag="xht")
            xlt0 = xtp.tile([128, NE, TC], FP8, tag="xlt")
            nc.sync.dma_start(out=xht0, in_=xh8[0])
            nc.sync.dma_start(out=xlt0, in_=xl8[0])

            wqh_sb = wts.tile([128, NE, 512], FP8, tag="wqh")
            wql_sb = wts.tile([128, NE, 512], FP8, tag="wql")
            wkh_sb = wts.tile([128, NE, HD], FP8, tag="wkh")
            wkl_sb = wts.tile([128, NE, HD], FP8, tag="wkl")
            wvh_sb = wts.tile([128, NE, HD], FP8, tag="wvh")
            wvl_sb = wts.tile([128, NE, HD], FP8, tag="wvl")
            nc.sync.dma_start(out=wqh_sb, in_=wqh[:])
            nc.sync.dma_start(out=wql_sb, in_=wql[:])
            nc.sync.dma_start(out=wkh_sb, in_=wkh[:])
            nc.sync.dma_start(out=wkl_sb, in_=wkl[:])
            nc.sync.dma_start(out=wvh_sb, in_=wvh[:])
            nc.sync.dma_start(out=wvl_sb, in_=wvl[:])

            # ---- small constants + rope tables (needed mid-chunk-0) ----
            masks_sb = cst.tile([128, 4, 512], BF16, tag="masks")
            masksn_sb = cst.tile([128, 256], BF16, tag="masksn")
            ones_sb = cst.tile([128, 128], BF16, tag="ones")
            eye_sb = cst.tile([128, 128], BF16, tag="eye")
            eps_sb = cst.tile([128, 1], F32, tag="eps")
            nc.sync.dma_start(out=ones_sb, in_=ones_in[:])
            nc.vector.memset(eps_sb, EPS * WSC * WSC)

            crep_sb = wts.tile([128, T], BF16, tag="crep")
            ssgn_sb = wts.tile([128, T], BF16, tag="ssgn")
            nc.sync.dma_start(out=crep_sb, in_=crep[:])
            nc.sync.dma_start(out=ssgn_sb, in_=ssgn[:])

            # ---- streams only needed in phase 1b / 2 ----
            xg_sb = wts.tile([GATE_C, T], BF16, tag="xg")
            vef_sb = wts.tile([HD, T], BF16, tag="vef")
            wg_sb = wts.tile([GATE_C, 128], BF16, tag="wg")
            nc.sync.dma_start(out=xg_sb, in_=xg[:])
            nc.sync.dma_start(out=vef_sb, in_=veT[:])
            nc.sync.dma_start(out=wg_sb, in_=wg[:])
            nc.sync.dma_start(out=eye_sb, in_=eye_in[:])
            nc.sync.dma_start(out=masks_sb, in_=m_in.rearrange("m p f -> p m f"))
            nc.sync.dma_start(out=masksn_sb, in_=mn_in[:])

            # ---- persistent results (yT overwrites qT slices in phase 2) ----
            qyT_sb = res.tile([128, 4, T], BF16, tag="qyT")
            kT_sb = res.tile([128, T], BF16, tag="kT")
            vn_sb = res.tile([128, NKB, HD], BF16, tag="vn")
            vraw_sb = res.tile([128, NTC, TC], BF16, tag="vraw")

            def proj_chains(ps, wh_sb, wl_sb, xh_t, xl_t, dsl):
                """3-chain fp8 DoubleRow projection into PSUM ps."""
                n = 3 * NE2
                i = 0
                for w_sb, x_t in ((wh_sb, xh_t), (wl_sb, xh_t), (wh_sb, xl_t)):
                    for e2 in range(NE2):
                        nc.tensor.matmul(
                            ps, w_sb[:, 2 * e2:2 * e2 + 2, dsl], x_t[:, 2 * e2:2 * e2 + 2, :],
                            start=(i == 0), stop=(i == n - 1), perf_mode=DR)
                        i += 1

            def emit_attn(hp, qs):
                h2 = slice(2 * hp, 2 * hp + 2)
                q0 = qs * TC
                kb0 = max(0, 2 * qs - 8)
                kb1 = 2 * qs + 2
                o_ps = p_or.tile([128, 512], F32, tag="or")
                r_ps = p_or.tile([128, 512], F32, tag="or")
                for kb in range(kb0, kb1):
                    if kb == 2 * qs + 1:
                        # diag end: only q-high halves live; contributes to the
                        # (h, q-high) sub-columns, start=False (never first)
                        rhs_hi = qyT_sb[:, h2, q0 + 128:q0 + 256]
                        s_n = p_s.tile([128, 256], F32, tag="s")
                        nc.tensor.matmul(s_n, kT_sb[:, kb * 128:(kb + 1) * 128],
                                         rhs_hi, start=True, stop=True)
                        pt_n = ptp.tile([128, 256], BF16, tag="pt")
                        nc.scalar.activation(pt_n, s_n, AF.Exp, scale=float(SCALE))
                        nc.vector.tensor_tensor(pt_n, pt_n, masksn_sb, OP.mult)
                        o_v = o_ps.rearrange("p (h q) -> p h q", h=2)
                        r_v = r_ps.rearrange("p (h q) -> p h q", h=2)
                        nc.tensor.matmul(o_v[:, :, 128:256], vn_sb[:, kb, :], pt_n,
                                         start=False, stop=True, skip_group_check=True)
                        nc.tensor.matmul(r_v[:, :, 128:256], ones_sb, pt_n,
                                         start=False, stop=True, skip_group_check=True)
                        continue
                    s_ps = p_s.tile([128, 512], F32, tag="s")
                    nc.tensor.matmul(s_ps,
                                     kT_sb[:, kb * 128:(kb + 1) * 128],
                                     qyT_sb[:, h2, q0:q0 + TC],
                                     start=True, stop=True)
                    pt = ptp.tile([128, 512], BF16, tag="pt")
                    nc.scalar.activation(pt, s_ps, AF.Exp, scale=float(SCALE))
                    mi = None
                    if kb == 2 * qs:
                        mi = 0
                    elif qs >= 4 and kb == kb0:
                        mi = 2
                    elif qs >= 4 and kb == kb0 + 1:
                        mi = 3
                    if mi is not None:
                        nc.vector.tensor_tensor(pt, pt, masks_sb[:, mi, :], OP.mult)
                    nc.tensor.matmul(o_ps, vn_sb[:, kb, :], pt,
                                     start=(kb == kb0), stop=False, skip_group_check=True)
                    nc.tensor.matmul(r_ps, ones_sb, pt,
                                     start=(kb == kb0), stop=False, skip_group_check=True)
                rr = wk2.tile([128, 512], F32, tag="rr")
                nc.vector.reciprocal(rr, r_ps)
                nc.vector.tensor_mul(qyT_sb[:, h2, q0:q0 + TC], o_ps, rr)

            # ================= phase 1: projections + rms/rope (Sqrt only) ====
            for tcix in range(NTC):
                ts = tcix * TC
                if tcix == 0:
                    xh_t, xl_t = xht0, xlt0
                else:
                    xh_t = xtp.tile([128, NE, TC], FP8, tag="xht")
                    xl_t = xtp.tile([128, NE, TC], FP8, tag="xlt")
                    nc.sync.dma_start(out=xh_t, in_=xh8[tcix])
                    nc.sync.dma_start(out=xl_t, in_=xl8[tcix])
                c_sl = crep_sb[:, ts:ts + TC]
                s_sl = ssgn_sb[:, ts:ts + TC]

                # projections + rms + rope; sumsq paired per 2 srcs so each
                # Sqrt covers two sources (fewer act-table switches)
                srcs = [("q", 0), ("q", 1), ("q", 2), ("q", 3), ("k", 0)]
                chunk_qraws = []
                ss_pair = None
                rr_pair = None
                for i, (kind, h) in enumerate(srcs):
                    ps = p_q.tile([128, TC], F32, tag="q")
                    if kind == "q":
                        proj_chains(ps, wqh_sb, wql_sb, xh_t, xl_t,
                                    slice(h * 128, (h + 1) * 128))
                    else:
                        proj_chains(ps, wkh_sb, wkl_sb, xh_t, xl_t, slice(0, HD))
                    qraw = qrp.tile([128, TC], BF16, tag="qraw")
                    nc.vector.tensor_copy(qraw, ps)
                    chunk_qraws.append(qraw)
                    sq = wk1.tile([128, TC], BF16, tag="sq")
                    nc.vector.tensor_mul(sq, qraw, qraw)
                    half = i % 2
                    if half == 0:
                        ss_pair = p_sm.tile([128, 512], F32, tag="small")
                        rr_pair = wk2.tile([128, 512], F32, tag="rrms")
                    nc.tensor.matmul(ss_pair[:, half * TC:(half + 1) * TC],
                                     ones_sb, sq, start=True, stop=True)
                    if half == 1 or i == 4:
                        wd = 512 if half == 1 else 256
                        nc.scalar.activation(rr_pair[:, 0:wd], ss_pair[:, 0:wd],
                                             AF.Sqrt, bias=eps_sb, scale=1.0 / HD)
                        nc.vector.reciprocal(rr_pair[:, 0:wd], rr_pair[:, 0:wd])
                        done = [i - 1, i] if half == 1 else [i]
                        for ii in done:
                            kind2, h2 = srcs[ii]
                            qraw2 = chunk_qraws[ii]
                            rrms = rr_pair[:, (ii % 2) * TC:(ii % 2 + 1) * TC]
                            qsw = wk1.tile([128, TC], BF16, tag="qsw")
                            nc.sync.dma_start(out=qsw[0:64, :], in_=qraw2[64:128, :])
                            nc.sync.dma_start(out=qsw[64:128, :], in_=qraw2[0:64, :])
                            tA = wk1.tile([128, TC], BF16, tag="tA")
                            tB = wk1.tile([128, TC], BF16, tag="tB")
                            nc.vector.tensor_mul(tA, qraw2, c_sl)
                            nc.gpsimd.tensor_tensor(tB, qsw, s_sl, OP.mult)
                            nc.vector.tensor_add(tA, tA, tB)
                            dest = (qyT_sb[:, h2, ts:ts + TC] if kind2 == "q"
                                    else kT_sb[:, ts:ts + TC])
                            nc.vector.tensor_mul(dest, tA, rrms)

                # v: projection only; gated ve mixing happens in phase 1b
                ps_v = p_q.tile([128, TC], F32, tag="q")
                proj_chains(ps_v, wvh_sb, wvl_sb, xh_t, xl_t, slice(0, HD))
                nc.vector.tensor_copy(vraw_sb[:, tcix, :], ps_v)

            # ================= phase 1b: gate + v mix + transpose (Exp table) ==
            for tcix in range(NTC):
                ts = tcix * TC
                # gate via exp: g = 1/(1+exp(-u)); the 2x (and v's 64x) folds
                # into the STT scalar
                g_ps = p_sm.tile([128, TC], F32, tag="small")
                nc.tensor.matmul(g_ps, wg_sb, xg_sb[:, ts:ts + TC], start=True, stop=True)
                g_rep = wk2.tile([128, TC], F32, tag="grep")
                nc.scalar.activation(g_rep, g_ps, AF.Exp, scale=-1.0)
                nc.vector.tensor_scalar_add(g_rep, g_rep, 1.0)
                nc.vector.reciprocal(g_rep, g_rep)
                tv = wk1.tile([128, TC], BF16, tag="tA")
                nc.gpsimd.tensor_tensor(tv, vef_sb[:, ts:ts + TC], g_rep, OP.mult)
                vt = wk1.tile([128, TC], BF16, tag="tB")
                nc.vector.scalar_tensor_tensor(vt, tv, 2.0 * WSC, vraw_sb[:, tcix, :],
                                               OP.mult, OP.add)
                for tb in range(TC // 128):
                    tp_ps = p_sm.tile([128, 128], BF16, tag="small")
                    nc.tensor.transpose(tp_ps, vt[:, tb * 128:(tb + 1) * 128], eye_sb)
                    nc.vector.tensor_copy(vn_sb[:, tcix * 2 + tb, :], tp_ps)

            # ================= phase 2: windowed attention (head-paired) =======
            for qs in range(NTC):
                for hp in range(2):
                    emit_attn(hp, qs)

            # ================= phase 3: out = y @ Wo (row-parallel partial) ====
            for os_ in range(4):
                wo_sl = wop.tile([128, 4, 512], BF16, tag="wo")
                nc.sync.dma_start(
                    out=wo_sl,
                    in_=wo.rearrange("(h d) o -> d h o", d=128)[:, :, os_ * 512:(os_ + 1) * 512],
                )
                for tt in range(T // 128):
                    pool3, tag3 = (p_s, "s") if tt % 2 == 0 else (p_or, "or")
                    po = pool3.tile([128, 512], F32, tag=tag3)
                    for h in range(4):
                        nc.tensor.matmul(po, qyT_sb[:, h, tt * 128:(tt + 1) * 128],
                                         wo_sl[:, h, :], start=(h == 0), stop=(h == 3))
                    stage = stg.tile([128, 512], BF16, tag="stage")
                    if tt % 2 == 0:
                        nc.vector.tensor_copy(stage, po)
                    else:
                        nc.scalar.copy(stage, po)
                    nc.sync.dma_start(
                        out=out[tt * 128:(tt + 1) * 128, os_ * 512:(os_ + 1) * 512],
                        in_=stage)

    nc.compile()
    return nc


def _masks():
    jj = np.arange(128)[:, None]
    ii = np.arange(128)[None, :]
    tri_d = (jj <= ii).astype(np.float32)   # diag block: keep j <= i
    tri_f = (jj >= ii).astype(np.float32)   # far block: keep j >= i - WIN
    one = np.ones((128, 128), np.float32)
    zero = np.zeros((128, 128), np.float32)
    m0 = np.concatenate([tri_d, one], 1)
    m1 = np.concatenate([zero, tri_d], 1)
    m2 = np.concatenate([tri_f, zero], 1)
    m3 = np.concatenate([one, tri_f], 1)
    base = np.ascontiguousarray(np.tile(np.stack([m0, m1, m2, m3]), (1, 1, 2)))
    mn = np.ascontiguousarray(np.concatenate([tri_d, tri_d], 1))
    return base, mn


def _hilo(a, scale=1.0):
    import ml_dtypes
    F8 = ml_dtypes.float8_e4m3
    s = (a * scale).astype(np.float32)
    h = s.astype(F8)
    l = (s - h.astype(np.float32)).astype(F8)
    return np.ascontiguousarray(h), np.ascontiguousarray(l)


def _pack_x(a):
    # [E, T] -> chunk-major [NTC, 128, NE, TC] (partition p owns row 128e+p)
    return np.ascontiguousarray(
        a.reshape(NE, 128, NTC, TC).transpose(2, 1, 0, 3))


def _pack_w(a):
    # [E, D] -> partition-major [128, NE, D]
    return np.ascontiguousarray(a.reshape(NE, 128, -1).transpose(1, 0, 2))


def kernel(**inputs):
    import ml_dtypes
    from concourse.bass_utils import run_bass_kernel_spmd

    BF = ml_dtypes.bfloat16

    if "nc" not in _CACHE:
        _CACHE["nc"] = _build_program()
    nc = _CACHE["nc"]

    x = np.asarray(inputs["x"], np.float32)
    ve = np.asarray(inputs["ve"], np.float32)
    cos = np.asarray(inputs["cos"], np.float32)
    sin = np.asarray(inputs["sin"], np.float32)
    Wq = np.asarray(inputs["Wq"], np.float32)
    Wk = np.asarray(inputs["Wk"], np.float32)
    Wv = np.asarray(inputs["Wv"], np.float32)
    Wo = np.asarray(inputs["Wo"], np.float32)
    Wg = np.asarray(inputs["Wg"], np.float32)

    crep = np.ascontiguousarray(np.concatenate([cos.T, cos.T], 0)).astype(BF)
    ssgn = np.ascontiguousarray(np.concatenate([sin.T, -sin.T], 0)).astype(BF)
    masks, masksn = _masks()
    masks = masks.astype(BF)
    masksn = masksn.astype(BF)
    ones128 = np.ones((128, 128), BF)
    eye128 = np.eye(128, dtype=BF)

    in_maps = []
    for c in range(8):
        b, g = divmod(c, 4)
        xT = np.ascontiguousarray(x[b].T)
        xh, xl = _hilo(xT)
        wq_h, wq_l = _hilo(Wq[:, g * 512:(g + 1) * 512], WSC)
        wk_h, wk_l = _hilo(Wk[:, g * HD:(g + 1) * HD], WSC)
        wv_h, wv_l = _hilo(Wv[:, g * HD:(g + 1) * HD], WSC)
        xh, xl = _pack_x(xh), _pack_x(xl)
        wq_h, wq_l = _pack_w(wq_h), _pack_w(wq_l)
        wk_h, wk_l = _pack_w(wk_h), _pack_w(wk_l)
        wv_h, wv_l = _pack_w(wv_h), _pack_w(wv_l)
        in_maps.append({
            "xh8": xh,
            "xl8": xl,
            "xg": np.ascontiguousarray(xT[:GATE_C]).astype(BF),
            "veT": np.ascontiguousarray(ve[b, :, g * HD:(g + 1) * HD].T).astype(BF),
            "crep": crep,
            "ssgn": ssgn,
            "wqh": wq_h, "wql": wq_l,
            "wkh": wk_h, "wkl": wk_l,
            "wvh": wv_h, "wvl": wv_l,
            "wg": np.ascontiguousarray(np.repeat(Wg[:, g:g + 1], 128, 1)).astype(BF),
            "wo": np.ascontiguousarray(Wo[g * 512:(g + 1) * 512, :] / WSC).astype(BF),
            "m_in": masks,
            "mn_in": masksn,
            "ones_in": ones128,
            "eye_in": eye128,
        })

    res = run_bass_kernel_spmd(nc, in_maps, core_ids=list(range(8)))
    parts = [np.asarray(res.results[c]["out"]).astype(np.float32) for c in range(8)]
    out = np.stack([parts[0] + parts[1] + parts[2] + parts[3],
                    parts[4] + parts[5] + parts[6] + parts[7]])
    return out.astype(np.float32)


# revision 13
# speedup vs baseline: 1.1064x; 1.0162x over previous
"""Sliding-window causal GQA self-attention (B=2, T=2048, 16 q-heads, 4 kv-heads,
head_dim=128, window=1024) on 8 trn2 NeuronCores.

Sharding: core = (batch b, kv-group g) -> 4 query heads + 1 kv head, full T.
Wo is row-parallel; each core emits a [T, 2048] bf16 partial that the host
upcasts and sums per batch (the unshard step for the row-parallel layout).

Precision plan (gate is rel-err < 2e-2 vs f32 reference; this lands ~4e-3):
  - QKV projections run as fp8e4m3 DoubleRow matmuls (2 contraction rows per
    partition, 0.5 PE cycles/row = 4x f32 rate) in an error-compensated
    3-chain form: x = xh + xl (host hi/lo split), W = (Wh + Wl)/64 (host
    split, x64 pre-scale keeps W out of the fp8 subnormal range), computing
    xh@Wh + xl@Wh + xh@Wl (the xl@Wl term is ~1e-4 relative and dropped).
    The 64x output scale cancels inside RMS-norm for q/k (bias = eps*64^2)
    and is folded into Wo on the host for the v path.
  - Everything else is bf16 (1 PE cycle/row, 2x DVE mode, half DMA): rope
    tables, masks, pt=exp(S), V, y, Wo, output. f32 only in PSUM, RMS-norm
    scales, softmax reciprocals, and the ve gate.

Device dataflow:
  phase 1: qT/kT/vT projections (3-chain fp8 DR), RoPE (half-swap DMA +
           [c;c], [s;-s] tables), RMS-norm via squared-input all-ones-matmul
           replicated sum; raw v^T parked in SBUF. ACT runs Sqrt only.
  phase 1b: gate sigmoid via Exp (bf16 x-stripe matmul), v = v_raw + gated ve,
           PE-transpose of v^T into natural V. One Exp act-table load that
           phase 2 reuses (act-table thrash was ~27 loads x 1.3us).
  phase 2: S^T = K^T.T @ Q^T per 128-key block x 256-query super; ACT exp
           (scale fused) -> bf16; 0/1 triangle masks for window edges; PV and
           all-ones rowsum accumulated in PSUM; normalize on evacuation
           (y^T overwrites the dead q^T slice).
  phase 3: out[t, o] = sum_h yT_h^T @ Wo_h, Wo streamed per 512-col slice.
"""

import numpy as np

B, T, E = 2, 2048, 2048
NH, NKV, HD = 16, 4, 128
GATE_C = 32
WIN = 1024
EPS = 1e-6
NE = E // 128          # 16 contraction chunks
NE2 = NE // 2          # 8 fp8 DoubleRow pair-chunks
TC = 256               # phase-1 token chunk (= q-super width)
NTC = T // TC          # 8
NKB = T // 128         # 16 key blocks
SCALE = 1.0 / np.sqrt(HD)
WSC = 64.0             # host pre-scale on Wq/Wk/Wv for fp8 range

_CACHE = {}


def _build_program():
    import concourse.bacc as bacc
    import concourse.mybir as mybir
    import concourse.tile as tile

    F32, BF16, FP8 = mybir.dt.float32, mybir.dt.bfloat16, mybir.dt.float8e4
    AF = mybir.ActivationFunctionType
    OP = mybir.AluOpType
    DR = mybir.MatmulPerfMode.DoubleRow

    nc = bacc.Bacc("TRN2", target_bir_lowering=False, debug=False, num_devices=8)

    # x and weights are host-packed chunk-major/partition-major so every DMA
    # reads >=2KB contiguous per partition (short runs pay 2x DMA latency)
    xh8 = nc.dram_tensor("xh8", [NTC, 128, NE, TC], FP8, kind="ExternalInput")
    xl8 = nc.dram_tensor("xl8", [NTC, 128, NE, TC], FP8, kind="ExternalInput")
    xg = nc.dram_tensor("xg", [GATE_C, T], BF16, kind="ExternalInput")
    veT = nc.dram_tensor("veT", [HD, T], BF16, kind="ExternalInput")
    crep = nc.dram_tensor("crep", [128, T], BF16, kind="ExternalInput")
    ssgn = nc.dram_tensor("ssgn", [128, T], BF16, kind="ExternalInput")
    wqh = nc.dram_tensor("wqh", [128, NE, 512], FP8, kind="ExternalInput")
    wql = nc.dram_tensor("wql", [128, NE, 512], FP8, kind="ExternalInput")
    wkh = nc.dram_tensor("wkh", [128, NE, HD], FP8, kind="ExternalInput")
    wkl = nc.dram_tensor("wkl", [128, NE, HD], FP8, kind="ExternalInput")
    wvh = nc.dram_tensor("wvh", [128, NE, HD], FP8, kind="ExternalInput")
    wvl = nc.dram_tensor("wvl", [128, NE, HD], FP8, kind="ExternalInput")
    wg = nc.dram_tensor("wg", [GATE_C, 128], BF16, kind="ExternalInput")
    wo = nc.dram_tensor("wo", [512, E], BF16, kind="ExternalInput")
    m_in = nc.dram_tensor("m_in", [4, 128, 512], BF16, kind="ExternalInput")
    mn_in = nc.dram_tensor("mn_in", [128, 256], BF16, kind="ExternalInput")
    ones_in = nc.dram_tensor("ones_in", [128, 128], BF16, kind="ExternalInput")
    eye_in = nc.dram_tensor("eye_in", [128, 128], BF16, kind="ExternalInput")
    out = nc.dram_tensor("out", [T, E], BF16, kind="ExternalOutput")

    with tile.TileContext(nc) as tc:
        from contextlib import ExitStack
        with ExitStack() as ctx:
            cst = ctx.enter_context(tc.tile_pool(name="cst", bufs=1))
            wts = ctx.enter_context(tc.tile_pool(name="wts", bufs=1))
            xtp = ctx.enter_context(tc.tile_pool(name="xtp", bufs=2))
            res = ctx.enter_context(tc.tile_pool(name="res", bufs=1))
            qrp = ctx.enter_context(tc.tile_pool(name="qrp", bufs=5))
            wk1 = ctx.enter_context(tc.tile_pool(name="wk1", bufs=4))
            wk2 = ctx.enter_context(tc.tile_pool(name="wk2", bufs=2))
            ptp = ctx.enter_context(tc.tile_pool(name="ptp", bufs=4))
            wop = ctx.enter_context(tc.tile_pool(name="wop", bufs=2))
            stg = ctx.enter_context(tc.tile_pool(name="stg", bufs=4))
            p_q = ctx.enter_context(tc.tile_pool(name="p_q", bufs=2, space="PSUM"))
            p_sm = ctx.enter_context(tc.tile_pool(name="p_sm", bufs=1, space="PSUM"))
            p_s = ctx.enter_context(tc.tile_pool(name="p_s", bufs=3, space="PSUM"))
            p_or = ctx.enter_context(tc.tile_pool(name="p_or", bufs=2, space="PSUM"))

            # ---- chunk-0 x + first weights FIRST so compute starts early ----
            xht0 = xtp.tile([128, NE, TC], FP8, tag="xht")
            xlt0 = xtp.tile([128, NE, TC], FP8, tag="xlt")
            nc.sync.dma_start(out=xht0, in_=xh8[0])
            nc.sync.dma_start(out=xlt0, in_=xl8[0])

            wqh_sb = wts.tile([128, NE, 512], FP8, tag="wqh")
            wql_sb = wts.tile([128, NE, 512], FP8, tag="wql")
            wkh_sb = wts.tile([128, NE, HD], FP8, tag="wkh")
            wkl_sb = wts.tile([128, NE, HD], FP8, tag="wkl")
            wvh_sb = wts.tile([128, NE, HD], FP8, tag="wvh")
            wvl_sb = wts.tile([128, NE, HD], FP8, tag="wvl")
            nc.sync.dma_start(out=wqh_sb[:, 0:8, :], in_=wqh[:, 0:8, :])
            nc.sync.dma_start(out=wqh_sb[:, 8:16, :], in_=wqh[:, 8:16, :])
            nc.sync.dma_start(out=wql_sb[:, 0:8, :], in_=wql[:, 0:8, :])
            nc.sync.dma_start(out=wql_sb[:, 8:16, :], in_=wql[:, 8:16, :])
            nc.sync.dma_start(out=wkh_sb, in_=wkh[:])
            nc.sync.dma_start(out=wkl_sb, in_=wkl[:])
            nc.sync.dma_start(out=wvh_sb, in_=wvh[:])
            nc.sync.dma_start(out=wvl_sb, in_=wvl[:])

            # ---- small constants + rope tables (needed mid-chunk-0) ----
            masks_sb = cst.tile([128, 4, 512], BF16, tag="masks")
            masksn_sb = cst.tile([128, 256], BF16, tag="masksn")
            ones_sb = cst.tile([128, 128], BF16, tag="ones")
            eye_sb = cst.tile([128, 128], BF16, tag="eye")
            eps_sb = cst.tile([128, 1], F32, tag="eps")
            nc.sync.dma_start(out=ones_sb, in_=ones_in[:])
            nc.vector.memset(eps_sb, EPS * WSC * WSC)

            crep_sb = wts.tile([128, T], BF16, tag="crep")
            ssgn_sb = wts.tile([128, T], BF16, tag="ssgn")
            nc.sync.dma_start(out=crep_sb, in_=crep[:])
            nc.sync.dma_start(out=ssgn_sb, in_=ssgn[:])

            # ---- streams only needed in phase 1b / 2: DMAs emitted later ----
            xg_sb = wts.tile([GATE_C, T], BF16, tag="xg")
            vef_sb = wts.tile([HD, T], BF16, tag="vef")
            wg_sb = wts.tile([GATE_C, 128], BF16, tag="wg")

            def emit_late_dmas():
                nc.sync.dma_start(out=xg_sb, in_=xg[:])
                nc.sync.dma_start(out=vef_sb, in_=veT[:])
                nc.sync.dma_start(out=wg_sb, in_=wg[:])
                nc.sync.dma_start(out=eye_sb, in_=eye_in[:])
                nc.sync.dma_start(out=masks_sb, in_=m_in.rearrange("m p f -> p m f"))
                nc.sync.dma_start(out=masksn_sb, in_=mn_in[:])

            # ---- persistent results (yT overwrites qT slices in phase 2) ----
            qyT_sb = res.tile([128, 4, T], BF16, tag="qyT")
            kT_sb = res.tile([128, T], BF16, tag="kT")
            vn_sb = res.tile([128, NKB, HD], BF16, tag="vn")
            vraw_sb = res.tile([128, NTC, TC], BF16, tag="vraw")

            def proj_chains(ps, wh_sb, wl_sb, xh_t, xl_t, dsl):
                """3-chain fp8 DoubleRow projection into PSUM ps."""
                n = 3 * NE2
                i = 0
                for w_sb, x_t in ((wh_sb, xh_t), (wl_sb, xh_t), (wh_sb, xl_t)):
                    for e2 in range(NE2):
                        nc.tensor.matmul(
                            ps, w_sb[:, 2 * e2:2 * e2 + 2, dsl], x_t[:, 2 * e2:2 * e2 + 2, :],
                            start=(i == 0), stop=(i == n - 1), perf_mode=DR)
                        i += 1

            def emit_attn(hp, qs):
                h2 = slice(2 * hp, 2 * hp + 2)
                q0 = qs * TC
                kb0 = max(0, 2 * qs - 8)
                kb1 = 2 * qs + 2
                o_ps = p_or.tile([128, 512], F32, tag="or")
                r_ps = p_or.tile([128, 512], F32, tag="or")
                for kb in range(kb0, kb1):
                    if kb == 2 * qs + 1:
                        # diag end: only q-high halves live; contributes to the
                        # (h, q-high) sub-columns, start=False (never first)
                        rhs_hi = qyT_sb[:, h2, q0 + 128:q0 + 256]
                        s_n = p_s.tile([128, 256], F32, tag="s")
                        nc.tensor.matmul(s_n, kT_sb[:, kb * 128:(kb + 1) * 128],
                                         rhs_hi, start=True, stop=True)
                        pt_n = ptp.tile([128, 256], BF16, tag="pt")
                        nc.scalar.activation(pt_n, s_n, AF.Exp, scale=float(SCALE))
                        nc.vector.tensor_tensor(pt_n, pt_n, masksn_sb, OP.mult)
                        o_v = o_ps.rearrange("p (h q) -> p h q", h=2)
                        r_v = r_ps.rearrange("p (h q) -> p h q", h=2)
                        nc.tensor.matmul(o_v[:, :, 128:256], vn_sb[:, kb, :], pt_n,
                                         start=False, stop=True, skip_group_check=True)
                        nc.tensor.matmul(r_v[:, :, 128:256], ones_sb, pt_n,
                                         start=False, stop=True, skip_group_check=True)
                        continue
                    s_ps = p_s.tile([128, 512], F32, tag="s")
                    nc.tensor.matmul(s_ps,
                                     kT_sb[:, kb * 128:(kb + 1) * 128],
                                     qyT_sb[:, h2, q0:q0 + TC],
                                     start=True, stop=True)
                    pt = ptp.tile([128, 512], BF16, tag="pt")
                    nc.scalar.activation(pt, s_ps, AF.Exp, scale=float(SCALE))
                    mi = None
                    if kb == 2 * qs:
                        mi = 0
                    elif qs >= 4 and kb == kb0:
                        mi = 2
                    elif qs >= 4 and kb == kb0 + 1:
                        mi = 3
                    if mi is not None:
                        nc.vector.tensor_tensor(pt, pt, masks_sb[:, mi, :], OP.mult)
                    nc.tensor.matmul(o_ps, vn_sb[:, kb, :], pt,
                                     start=(kb == kb0), stop=False, skip_group_check=True)
                    nc.tensor.matmul(r_ps, ones_sb, pt,
                                     start=(kb == kb0), stop=False, skip_group_check=True)
                rr = wk2.tile([128, 512], F32, tag="rr")
                nc.vector.reciprocal(rr, r_ps)
                nc.vector.tensor_mul(qyT_sb[:, h2, q0:q0 + TC], o_ps, rr)

            # ================= phase 1: projections + rms/rope (Sqrt only) ====
            for tcix in range(NTC):
                ts = tcix * TC
                if tcix == 2:
                    emit_late_dmas()
                if tcix == 0:
                    xh_t, xl_t = xht0, xlt0
                else:
                    xh_t = xtp.tile([128, NE, TC], FP8, tag="xht")
                    xl_t = xtp.tile([128, NE, TC], FP8, tag="xlt")
                    nc.sync.dma_start(out=xh_t, in_=xh8[tcix])
                    nc.sync.dma_start(out=xl_t, in_=xl8[tcix])
                c_sl = crep_sb[:, ts:ts + TC]
                s_sl = ssgn_sb[:, ts:ts + TC]

                # projections + rms + rope; sumsq paired per 2 srcs so each
                # Sqrt covers two sources (fewer act-table switches)
                srcs = [("q", 0), ("q", 1), ("q", 2), ("q", 3), ("k", 0)]
                chunk_qraws = []
                ss_pair = None
                rr_pair = None
                for i, (kind, h) in enumerate(srcs):
                    ps = p_q.tile([128, TC], F32, tag="q")
                    if kind == "q":
                        proj_chains(ps, wqh_sb, wql_sb, xh_t, xl_t,
                                    slice(h * 128, (h + 1) * 128))
                    else:
                        proj_chains(ps, wkh_sb, wkl_sb, xh_t, xl_t, slice(0, HD))
                    qraw = qrp.tile([128, TC], BF16, tag="qraw")
                    nc.vector.tensor_copy(qraw, ps)
                    chunk_qraws.append(qraw)
                    sq = wk1.tile([128, TC], BF16, tag="sq")
                    nc.vector.tensor_mul(sq, qraw, qraw)
                    half = i % 2
                    if half == 0:
                        ss_pair = p_sm.tile([128, 512], F32, tag="small")
                        rr_pair = wk2.tile([128, 512], F32, tag="rrms")
                    nc.tensor.matmul(ss_pair[:, half * TC:(half + 1) * TC],
                                     ones_sb, sq, start=True, stop=True)
                    if half == 1 or i == 4:
                        wd = 512 if half == 1 else 256
                        nc.scalar.activation(rr_pair[:, 0:wd], ss_pair[:, 0:wd],
                                             AF.Sqrt, bias=eps_sb, scale=1.0 / HD)
                        nc.vector.reciprocal(rr_pair[:, 0:wd], rr_pair[:, 0:wd])
                        done = [i - 1, i] if half == 1 else [i]
                        for ii in done:
                            kind2, h2 = srcs[ii]
                            qraw2 = chunk_qraws[ii]
                            rrms = rr_pair[:, (ii % 2) * TC:(ii % 2 + 1) * TC]
                            qsw = wk1.tile([128, TC], BF16, tag="qsw")
                            nc.sync.dma_start(out=qsw[0:64, :], in_=qraw2[64:128, :])
                            nc.sync.dma_start(out=qsw[64:128, :], in_=qraw2[0:64, :])
                            tA = wk1.tile([128, TC], BF16, tag="tA")
                            tB = wk1.tile([128, TC], BF16, tag="tB")
                            nc.vector.tensor_mul(tA, qraw2, c_sl)
                            nc.gpsimd.tensor_tensor(tB, qsw, s_sl, OP.mult)
                            nc.vector.tensor_add(tA, tA, tB)
                            dest = (qyT_sb[:, h2, ts:ts + TC] if kind2 == "q"
                                    else kT_sb[:, ts:ts + TC])
                            nc.vector.tensor_mul(dest, tA, rrms)

                # v: projection only; gated ve mixing happens in phase 1b
                ps_v = p_q.tile([128, TC], F32, tag="q")
                proj_chains(ps_v, wvh_sb, wvl_sb, xh_t, xl_t, slice(0, HD))
                nc.vector.tensor_copy(vraw_sb[:, tcix, :], ps_v)

            # ======= phase 1b: gate + v mix + transpose (Exp table), =========
            # ======= interleaved with phase 2 so attention hides the DVE work
            def emit_vmix(tcix):
                ts = tcix * TC
                # gate via exp: g = 1/(1+exp(-u)); the 2x (and v's 64x) folds
                # into the STT scalar
                g_ps = p_sm.tile([128, TC], F32, tag="small")
                nc.tensor.matmul(g_ps, wg_sb, xg_sb[:, ts:ts + TC], start=True, stop=True)
                g_rep = wk2.tile([128, TC], F32, tag="grep")
                nc.scalar.activation(g_rep, g_ps, AF.Exp, scale=-1.0)
                nc.vector.tensor_scalar_add(g_rep, g_rep, 1.0)
                nc.vector.reciprocal(g_rep, g_rep)
                tv = wk1.tile([128, TC], BF16, tag="tA")
                nc.gpsimd.tensor_tensor(tv, vef_sb[:, ts:ts + TC], g_rep, OP.mult)
                vt = wk1.tile([128, TC], BF16, tag="tB")
                nc.vector.scalar_tensor_tensor(vt, tv, 2.0 * WSC, vraw_sb[:, tcix, :],
                                               OP.mult, OP.add)
                for tb in range(TC // 128):
                    tp_ps = p_sm.tile([128, 128], BF16, tag="small")
                    nc.tensor.transpose(tp_ps, vt[:, tb * 128:(tb + 1) * 128], eye_sb)
                    nc.vector.tensor_copy(vn_sb[:, tcix * 2 + tb, :], tp_ps)

            # ================= phase 2: windowed attention (head-paired) =======
            emit_vmix(0)
            for qs in range(NTC):
                if qs + 1 < NTC:
                    emit_vmix(qs + 1)
                for hp in range(2):
                    emit_attn(hp, qs)

            # ================= phase 3: out = y @ Wo (row-parallel partial) ====
            for os_ in range(4):
                wo_sl = wop.tile([128, 4, 512], BF16, tag="wo")
                nc.sync.dma_start(
                    out=wo_sl,
                    in_=wo.rearrange("(h d) o -> d h o", d=128)[:, :, os_ * 512:(os_ + 1) * 512],
                )
                for tt in range(T // 128):
                    pool3, tag3 = (p_s, "s") if tt % 2 == 0 else (p_or, "or")
                    po = pool3.tile([128, 512], F32, tag=tag3)
                    for h in range(4):
                        nc.tensor.matmul(po, qyT_sb[:, h, tt * 128:(tt + 1) * 128],
                                         wo_sl[:, h, :], start=(h == 0), stop=(h == 3))
                    stage = stg.tile([128, 512], BF16, tag="stage")
                    if tt % 2 == 0:
                        nc.vector.tensor_copy(stage, po)
                    else:
                        nc.scalar.copy(stage, po)
                    nc.sync.dma_start(
                        out=out[tt * 128:(tt + 1) * 128, os_ * 512:(os_ + 1) * 512],
                        in_=stage)

    nc.compile()
    return nc


def _masks():
    jj = np.arange(128)[:, None]
    ii = np.arange(128)[None, :]
    tri_d = (jj <= ii).astype(np.float32)   # diag block: keep j <= i
    tri_f = (jj >= ii).astype(np.float32)   # far block: keep j >= i - WIN
    one = np.ones((128, 128), np.float32)
    zero = np.zeros((128, 128), np.float32)
    m0 = np.concatenate([tri_d, one], 1)
    m1 = np.concatenate([zero, tri_d], 1)
    m2 = np.concatenate([tri_f, zero], 1)
    m3 = np.concatenate([one, tri_f], 1)
    base = np.ascontiguousarray(np.tile(np.stack([m0, m1, m2, m3]), (1, 1, 2)))
    mn = np.ascontiguousarray(np.concatenate([tri_d, tri_d], 1))
    return base, mn


def _hilo(a, scale=1.0):
    import ml_dtypes
    F8 = ml_dtypes.float8_e4m3
    s = (a * scale).astype(np.float32)
    h = s.astype(F8)
    l = (s - h.astype(np.float32)).astype(F8)
    return np.ascontiguousarray(h), np.ascontiguousarray(l)


def _pack_x(a):
    # [E, T] -> chunk-major [NTC, 128, NE, TC] (partition p owns row 128e+p)
    return np.ascontiguousarray(
        a.reshape(NE, 128, NTC, TC).transpose(2, 1, 0, 3))


def _pack_w(a):
    # [E, D] -> partition-major [128, NE, D]
    return np.ascontiguousarray(a.reshape(NE, 128, -1).transpose(1, 0, 2))


def kernel(**inputs):
    import ml_dtypes
    from concourse.bass_utils import run_bass_kernel_spmd

    BF = ml_dtypes.bfloat16

    if "nc" not in _CACHE:
        _CACHE["nc"] = _build_program()
    nc = _CACHE["nc"]

    x = np.asarray(inputs["x"], np.float32)
    ve = np.asarray(inputs["ve"], np.float32)
    cos = np.asarray(inputs["cos"], np.float32)
    sin = np.asarray(inputs["sin"], np.float32)
    Wq = np.asarray(inputs["Wq"], np.float32)
    Wk = np.asarray(inputs["Wk"], np.float32)
    Wv = np.asarray(inputs["Wv"], np.float32)
    Wo = np.asarray(inputs["Wo"], np.float32)
    Wg = np.asarray(inputs["Wg"], np.float32)

    crep = np.ascontiguousarray(np.concatenate([cos.T, cos.T], 0)).astype(BF)
    ssgn = np.ascontiguousarray(np.concatenate([sin.T, -sin.T], 0)).astype(BF)
    masks, masksn = _masks()
    masks = masks.astype(BF)
    masksn = masksn.astype(BF)
    ones128 = np.ones((128, 128), BF)
    eye128 = np.eye(128, dtype=BF)

    in_maps = []
    for c in range(8):
        b, g = divmod(c, 4)
        xT = np.ascontiguousarray(x[b].T)
        xh, xl = _hilo(xT)
        wq_h, wq_l = _hilo(Wq[:, g * 512:(g + 1) * 512], WSC)
        wk_h, wk_l = _hilo(Wk[:, g * HD:(g + 1) * HD], WSC)
        wv_h, wv_l = _hilo(Wv[:, g * HD:(g + 1) * HD], WSC)
        xh, xl = _pack_x(xh), _pack_x(xl)
        wq_h, wq_l = _pack_w(wq_h), _pack_w(wq_l)
        wk_h, wk_l = _pack_w(wk_h), _pack_w(wk_l)
        wv_h, wv_l = _pack_w(wv_h), _pack_w(wv_l)
        in_maps.append({
            "xh8": xh,
            "xl8": xl,
            "xg": np.ascontiguousarray(xT[:GATE_C]).astype(BF),
            "veT": np.ascontiguousarray(ve[b, :, g * HD:(g + 1) * HD].T).astype(BF),
            "crep": crep,
            "ssgn": ssgn,
            "wqh": wq_h, "wql": wq_l,
            "wkh": wk_h, "wkl": wk_l,
            "wvh": wv_h, "wvl": wv_l,
            "wg": np.ascontiguousarray(np.repeat(Wg[:, g:g + 1], 128, 1)).astype(BF),
            "wo": np.ascontiguousarray(Wo[g * 512:(g + 1) * 512, :] / WSC).astype(BF),
            "m_in": masks,
            "mn_in": masksn,
            "ones_in": ones128,
            "eye_in": eye128,
        })

    res = run_bass_kernel_spmd(nc, in_maps, core_ids=list(range(8)))
    parts = [np.asarray(res.results[c]["out"]).astype(np.float32) for c in range(8)]
    out = np.stack([parts[0] + parts[1] + parts[2] + parts[3],
                    parts[4] + parts[5] + parts[6] + parts[7]])
    return out.astype(np.float32)


# revision 16
# speedup vs baseline: 1.1150x; 1.0078x over previous
"""Sliding-window causal GQA self-attention (B=2, T=2048, 16 q-heads, 4 kv-heads,
head_dim=128, window=1024) on 8 trn2 NeuronCores.

Sharding: core = (batch b, kv-group g) -> 4 query heads + 1 kv head, full T.
Wo is row-parallel; each core emits a [T, 2048] bf16 partial that the host
upcasts and sums per batch (the unshard step for the row-parallel layout).

Precision plan (gate is rel-err < 2e-2 vs f32 reference; this lands ~4e-3):
  - QKV projections run as fp8e4m3 DoubleRow matmuls (2 contraction rows per
    partition, 0.5 PE cycles/row = 4x f32 rate) in an error-compensated
    3-chain form: x = xh + xl (host hi/lo split), W = (Wh + Wl)/64 (host
    split, x64 pre-scale keeps W out of the fp8 subnormal range), computing
    xh@Wh + xl@Wh + xh@Wl (the xl@Wl term is ~1e-4 relative and dropped).
    The 64x output scale cancels inside RMS-norm for q/k (bias = eps*64^2)
    and is folded into Wo on the host for the v path.
  - Everything else is bf16 (1 PE cycle/row, 2x DVE mode, half DMA): rope
    tables, masks, pt=exp(S), V, y, Wo, output. f32 only in PSUM, RMS-norm
    scales, softmax reciprocals, and the ve gate.

Device dataflow:
  phase 1: qT/kT/vT projections (3-chain fp8 DR), RoPE (half-swap DMA +
           [c;c], [s;-s] tables), RMS-norm via squared-input all-ones-matmul
           replicated sum; raw v^T parked in SBUF. ACT runs Sqrt only.
  phase 1b: gate sigmoid via Exp (bf16 x-stripe matmul), v = v_raw + gated ve,
           PE-transpose of v^T into natural V. One Exp act-table load that
           phase 2 reuses (act-table thrash was ~27 loads x 1.3us).
  phase 2: S^T = K^T.T @ Q^T per 128-key block x 256-query super; ACT exp
           (scale fused) -> bf16; 0/1 triangle masks for window edges; PV and
           all-ones rowsum accumulated in PSUM; normalize on evacuation
           (y^T overwrites the dead q^T slice).
  phase 3: out[t, o] = sum_h yT_h^T @ Wo_h, Wo streamed per 512-col slice.
"""

import numpy as np

B, T, E = 2, 2048, 2048
NH, NKV, HD = 16, 4, 128
GATE_C = 32
WIN = 1024
EPS = 1e-6
NE = E // 128          # 16 contraction chunks
NE2 = NE // 2          # 8 fp8 DoubleRow pair-chunks
TC = 256               # phase-1 token chunk (= q-super width)
NTC = T // TC          # 8
NKB = T // 128         # 16 key blocks
SCALE = 1.0 / np.sqrt(HD)
WSC = 64.0             # host pre-scale on Wq/Wk/Wv for fp8 range

_CACHE = {}


def _build_program():
    import concourse.bacc as bacc
    import concourse.mybir as mybir
    import concourse.tile as tile

    F32, BF16, FP8 = mybir.dt.float32, mybir.dt.bfloat16, mybir.dt.float8e4
    AF = mybir.ActivationFunctionType
    OP = mybir.AluOpType
    DR = mybir.MatmulPerfMode.DoubleRow

    nc = bacc.Bacc("TRN2", target_bir_lowering=False, debug=False, num_devices=8)

    # x and weights are host-packed chunk-major/partition-major so every DMA
    # reads >=2KB contiguous per partition (short runs pay 2x DMA latency)
    xh8 = nc.dram_tensor("xh8", [NTC, 128, NE, TC], FP8, kind="ExternalInput")
    xl8 = nc.dram_tensor("xl8", [NTC, 128, NE, TC], FP8, kind="ExternalInput")
    xg = nc.dram_tensor("xg", [GATE_C, T], BF16, kind="ExternalInput")
    veT = nc.dram_tensor("veT", [HD, T], BF16, kind="ExternalInput")
    crep = nc.dram_tensor("crep", [128, T], BF16, kind="ExternalInput")
    ssgn = nc.dram_tensor("ssgn", [128, T], BF16, kind="ExternalInput")
    wqh = nc.dram_tensor("wqh", [128, NE, 512], FP8, kind="ExternalInput")
    wql = nc.dram_tensor("wql", [128, NE, 512], FP8, kind="ExternalInput")
    wkh = nc.dram_tensor("wkh", [128, NE, HD], FP8, kind="ExternalInput")
    wkl = nc.dram_tensor("wkl", [128, NE, HD], FP8, kind="ExternalInput")
    wvh = nc.dram_tensor("wvh", [128, NE, HD], FP8, kind="ExternalInput")
    wvl = nc.dram_tensor("wvl", [128, NE, HD], FP8, kind="ExternalInput")
    wg = nc.dram_tensor("wg", [GATE_C, 128], BF16, kind="ExternalInput")
    wo = nc.dram_tensor("wo", [512, E], BF16, kind="ExternalInput")
    m_in = nc.dram_tensor("m_in", [4, 128, 512], BF16, kind="ExternalInput")
    mn_in = nc.dram_tensor("mn_in", [128, 256], BF16, kind="ExternalInput")
    ones_in = nc.dram_tensor("ones_in", [128, 128], BF16, kind="ExternalInput")
    eye_in = nc.dram_tensor("eye_in", [128, 128], BF16, kind="ExternalInput")
    out = nc.dram_tensor("out", [T, E], BF16, kind="ExternalOutput")

    with tile.TileContext(nc) as tc:
        from contextlib import ExitStack
        with ExitStack() as ctx:
            cst = ctx.enter_context(tc.tile_pool(name="cst", bufs=1))
            wts = ctx.enter_context(tc.tile_pool(name="wts", bufs=1))
            xtp = ctx.enter_context(tc.tile_pool(name="xtp", bufs=2))
            res = ctx.enter_context(tc.tile_pool(name="res", bufs=1))
            qrp = ctx.enter_context(tc.tile_pool(name="qrp", bufs=5))
            wk1 = ctx.enter_context(tc.tile_pool(name="wk1", bufs=4))
            wk2 = ctx.enter_context(tc.tile_pool(name="wk2", bufs=2))
            ptp = ctx.enter_context(tc.tile_pool(name="ptp", bufs=4))
            wop = ctx.enter_context(tc.tile_pool(name="wop", bufs=2))
            stg = ctx.enter_context(tc.tile_pool(name="stg", bufs=4))
            p_q = ctx.enter_context(tc.tile_pool(name="p_q", bufs=2, space="PSUM"))
            p_sm = ctx.enter_context(tc.tile_pool(name="p_sm", bufs=1, space="PSUM"))
            p_s = ctx.enter_context(tc.tile_pool(name="p_s", bufs=3, space="PSUM"))
            p_or = ctx.enter_context(tc.tile_pool(name="p_or", bufs=2, space="PSUM"))

            # ---- chunk-0 x + first weights FIRST so compute starts early ----
            # ordered by first use: xh/wqh/wql halves feed the first DR chains
            xht0 = xtp.tile([128, NE, TC], FP8, tag="xht")
            xlt0 = xtp.tile([128, NE, TC], FP8, tag="xlt")
            wqh_sb = wts.tile([128, NE, 512], FP8, tag="wqh")
            wql_sb = wts.tile([128, NE, 512], FP8, tag="wql")
            wkh_sb = wts.tile([128, NE, HD], FP8, tag="wkh")
            wkl_sb = wts.tile([128, NE, HD], FP8, tag="wkl")
            wvh_sb = wts.tile([128, NE, HD], FP8, tag="wvh")
            wvl_sb = wts.tile([128, NE, HD], FP8, tag="wvl")
            nc.sync.dma_start(out=xht0[:, 0:8, :], in_=xh8[0, :, 0:8, :])
            nc.sync.dma_start(out=wqh_sb[:, 0:8, :], in_=wqh[:, 0:8, :])
            nc.sync.dma_start(out=xht0[:, 8:16, :], in_=xh8[0, :, 8:16, :])
            nc.sync.dma_start(out=wqh_sb[:, 8:16, :], in_=wqh[:, 8:16, :])
            nc.sync.dma_start(out=wql_sb[:, 0:8, :], in_=wql[:, 0:8, :])
            nc.sync.dma_start(out=wql_sb[:, 8:16, :], in_=wql[:, 8:16, :])
            nc.sync.dma_start(out=xlt0, in_=xl8[0])
            nc.sync.dma_start(out=wkh_sb, in_=wkh[:])
            nc.sync.dma_start(out=wkl_sb, in_=wkl[:])
            nc.sync.dma_start(out=wvh_sb, in_=wvh[:])
            nc.sync.dma_start(out=wvl_sb, in_=wvl[:])

            # ---- small constants + rope tables (needed mid-chunk-0) ----
            masks_sb = cst.tile([128, 4, 512], BF16, tag="masks")
            masksn_sb = cst.tile([128, 256], BF16, tag="masksn")
            ones_sb = cst.tile([128, 128], BF16, tag="ones")
            eye_sb = cst.tile([128, 128], BF16, tag="eye")
            eps_sb = cst.tile([128, 1], F32, tag="eps")
            nc.sync.dma_start(out=ones_sb, in_=ones_in[:])
            nc.vector.memset(eps_sb, EPS * WSC * WSC)

            crep_sb = wts.tile([128, T], BF16, tag="crep")
            ssgn_sb = wts.tile([128, T], BF16, tag="ssgn")
            nc.sync.dma_start(out=crep_sb, in_=crep[:])
            nc.sync.dma_start(out=ssgn_sb, in_=ssgn[:])

            # ---- streams only needed in phase 1b / 2: DMAs emitted later ----
            xg_sb = wts.tile([GATE_C, T], BF16, tag="xg")
            vef_sb = wts.tile([HD, T], BF16, tag="vef")
            wg_sb = wts.tile([GATE_C, 128], BF16, tag="wg")

            def emit_late_dmas():
                nc.sync.dma_start(out=xg_sb, in_=xg[:])
                nc.sync.dma_start(out=vef_sb, in_=veT[:])
                nc.sync.dma_start(out=wg_sb, in_=wg[:])
                nc.sync.dma_start(out=eye_sb, in_=eye_in[:])
                nc.sync.dma_start(out=masks_sb, in_=m_in.rearrange("m p f -> p m f"))
                nc.sync.dma_start(out=masksn_sb, in_=mn_in[:])

            # ---- persistent results (yT overwrites qT slices in phase 2) ----
            qyT_sb = res.tile([128, 4, T], BF16, tag="qyT")
            kT_sb = res.tile([128, T], BF16, tag="kT")
            vn_sb = res.tile([128, NKB, HD], BF16, tag="vn")
            vraw_sb = res.tile([128, NTC, TC], BF16, tag="vraw")

            def proj_chains(ps, wh_sb, wl_sb, xh_t, xl_t, dsl):
                """3-chain fp8 DoubleRow projection into PSUM ps."""
                n = 3 * NE2
                i = 0
                for w_sb, x_t in ((wh_sb, xh_t), (wl_sb, xh_t), (wh_sb, xl_t)):
                    for e2 in range(NE2):
                        nc.tensor.matmul(
                            ps, w_sb[:, 2 * e2:2 * e2 + 2, dsl], x_t[:, 2 * e2:2 * e2 + 2, :],
                            start=(i == 0), stop=(i == n - 1), perf_mode=DR)
                        i += 1

            def emit_attn(hp, qs):
                h2 = slice(2 * hp, 2 * hp + 2)
                q0 = qs * TC
                kb0 = max(0, 2 * qs - 8)
                kb1 = 2 * qs + 2
                o_ps = p_or.tile([128, 512], F32, tag="or")
                r_ps = p_or.tile([128, 512], F32, tag="or")
                for kb in range(kb0, kb1):
                    if kb == 2 * qs + 1:
                        # diag end: only q-high halves live; contributes to the
                        # (h, q-high) sub-columns, start=False (never first)
                        rhs_hi = qyT_sb[:, h2, q0 + 128:q0 + 256]
                        s_n = p_s.tile([128, 256], F32, tag="s")
                        nc.tensor.matmul(s_n, kT_sb[:, kb * 128:(kb + 1) * 128],
                                         rhs_hi, start=True, stop=True)
                        pt_n = ptp.tile([128, 256], BF16, tag="pt")
                        nc.scalar.activation(pt_n, s_n, AF.Exp, scale=float(SCALE))
                        nc.vector.tensor_tensor(pt_n, pt_n, masksn_sb, OP.mult)
                        o_v = o_ps.rearrange("p (h q) -> p h q", h=2)
                        r_v = r_ps.rearrange("p (h q) -> p h q", h=2)
                        nc.tensor.matmul(o_v[:, :, 128:256], vn_sb[:, kb, :], pt_n,
                                         start=False, stop=True, skip_group_check=True)
                        nc.tensor.matmul(r_v[:, :, 128:256], ones_sb, pt_n,
                                         start=False, stop=True, skip_group_check=True)
                        continue
                    s_ps = p_s.tile([128, 512], F32, tag="s")
                    nc.tensor.matmul(s_ps,
                                     kT_sb[:, kb * 128:(kb + 1) * 128],
                                     qyT_sb[:, h2, q0:q0 + TC],
                                     start=True, stop=True)
                    pt = ptp.tile([128, 512], BF16, tag="pt")
                    nc.scalar.activation(pt, s_ps, AF.Exp, scale=float(SCALE))
                    mi = None
                    if kb == 2 * qs:
                        mi = 0
                    elif qs >= 4 and kb == kb0:
                        mi = 2
                    elif qs >= 4 and kb == kb0 + 1:
                        mi = 3
                    if mi is not None:
                        nc.vector.tensor_tensor(pt, pt, masks_sb[:, mi, :], OP.mult)
                    nc.tensor.matmul(o_ps, vn_sb[:, kb, :], pt,
                                     start=(kb == kb0), stop=False, skip_group_check=True)
                    nc.tensor.matmul(r_ps, ones_sb, pt,
                                     start=(kb == kb0), stop=False, skip_group_check=True)
                rr = wk2.tile([128, 512], F32, tag="rr")
                nc.vector.reciprocal(rr, r_ps)
                nc.vector.tensor_mul(qyT_sb[:, h2, q0:q0 + TC], o_ps, rr)

            # ================= phase 1: projections + rms/rope (Sqrt only) ====
            for tcix in range(NTC):
                ts = tcix * TC
                if tcix == 2:
                    emit_late_dmas()
                if tcix == 0:
                    xh_t, xl_t = xht0, xlt0
                else:
                    xh_t = xtp.tile([128, NE, TC], FP8, tag="xht")
                    xl_t = xtp.tile([128, NE, TC], FP8, tag="xlt")
                    nc.sync.dma_start(out=xh_t, in_=xh8[tcix])
                    nc.sync.dma_start(out=xl_t, in_=xl8[tcix])
                c_sl = crep_sb[:, ts:ts + TC]
                s_sl = ssgn_sb[:, ts:ts + TC]

                # projections + rms + rope; sumsq paired per 2 srcs so each
                # Sqrt covers two sources (fewer act-table switches)
                srcs = [("q", 0), ("q", 1), ("q", 2), ("q", 3), ("k", 0)]
                chunk_qraws = []
                ss_pair = None
                rr_pair = None
                for i, (kind, h) in enumerate(srcs):
                    ps = p_q.tile([128, TC], F32, tag="q")
                    if kind == "q":
                        proj_chains(ps, wqh_sb, wql_sb, xh_t, xl_t,
                                    slice(h * 128, (h + 1) * 128))
                    else:
                        proj_chains(ps, wkh_sb, wkl_sb, xh_t, xl_t, slice(0, HD))
                    qraw = qrp.tile([128, TC], BF16, tag="qraw")
                    nc.scalar.copy(qraw, ps)
                    chunk_qraws.append(qraw)
                    sq = wk1.tile([128, TC], BF16, tag="sq")
                    nc.vector.tensor_mul(sq, qraw, qraw)
                    half = i % 2
                    if half == 0:
                        ss_pair = p_sm.tile([128, 512], F32, tag="small")
                        rs_pair = wk2.tile([128, 512], F32, tag="rrms")
                        rr_pair = wk2.tile([128, 512], BF16, tag="rrmb")
                    nc.tensor.matmul(ss_pair[:, half * TC:(half + 1) * TC],
                                     ones_sb, sq, start=True, stop=True)
                    if half == 1 or i == 4:
                        wd = 512 if half == 1 else 256
                        nc.scalar.activation(rs_pair[:, 0:wd], ss_pair[:, 0:wd],
                                             AF.Sqrt, bias=eps_sb, scale=1.0 / HD)
                        with nc.allow_low_precision("rms scale, validated 4e-3"):
                            nc.vector.reciprocal(rr_pair[:, 0:wd], rs_pair[:, 0:wd])
                        done = [i - 1, i] if half == 1 else [i]
                        for ii in done:
                            kind2, h2 = srcs[ii]
                            qraw2 = chunk_qraws[ii]
                            rrms = rr_pair[:, (ii % 2) * TC:(ii % 2 + 1) * TC]
                            qsw = wk1.tile([128, TC], BF16, tag="qsw")
                            nc.sync.dma_start(out=qsw[0:64, :], in_=qraw2[64:128, :])
                            nc.sync.dma_start(out=qsw[64:128, :], in_=qraw2[0:64, :])
                            tA = wk1.tile([128, TC], BF16, tag="tA")
                            tB = wk1.tile([128, TC], BF16, tag="tB")
                            nc.vector.tensor_mul(tA, qraw2, c_sl)
                            nc.gpsimd.tensor_tensor(tB, qsw, s_sl, OP.mult)
                            nc.vector.tensor_add(tA, tA, tB)
                            dest = (qyT_sb[:, h2, ts:ts + TC] if kind2 == "q"
                                    else kT_sb[:, ts:ts + TC])
                            nc.vector.tensor_mul(dest, tA, rrms)

                # v: projection only; gated ve mixing happens in phase 1b
                ps_v = p_q.tile([128, TC], F32, tag="q")
                proj_chains(ps_v, wvh_sb, wvl_sb, xh_t, xl_t, slice(0, HD))
                nc.scalar.copy(vraw_sb[:, tcix, :], ps_v)

            # ======= phase 1b: gate + v mix + transpose (Exp table), =========
            # ======= interleaved with phase 2 so attention hides the DVE work
            def emit_vmix(tcix):
                ts = tcix * TC
                # gate via exp: g = 1/(1+exp(-u)); the 2x (and v's 64x) folds
                # into the STT scalar
                g_ps = p_sm.tile([128, TC], F32, tag="small")
                nc.tensor.matmul(g_ps, wg_sb, xg_sb[:, ts:ts + TC], start=True, stop=True)
                g_rep = wk2.tile([128, TC], F32, tag="grep")
                nc.scalar.activation(g_rep, g_ps, AF.Exp, scale=-1.0)
                nc.vector.tensor_scalar_add(g_rep, g_rep, 1.0)
                nc.vector.reciprocal(g_rep, g_rep)
                tv = wk1.tile([128, TC], BF16, tag="tA")
                nc.gpsimd.tensor_tensor(tv, vef_sb[:, ts:ts + TC], g_rep, OP.mult)
                vt = wk1.tile([128, TC], BF16, tag="tB")
                nc.vector.scalar_tensor_tensor(vt, tv, 2.0 * WSC, vraw_sb[:, tcix, :],
                                               OP.mult, OP.add)
                for tb in range(TC // 128):
                    tp_ps = p_sm.tile([128, 128], BF16, tag="small")
                    nc.tensor.transpose(tp_ps, vt[:, tb * 128:(tb + 1) * 128], eye_sb)
                    nc.vector.tensor_copy(vn_sb[:, tcix * 2 + tb, :], tp_ps)

            # ================= phase 2: windowed attention (head-paired) =======
            emit_vmix(0)
            for qs in range(NTC):
                if qs + 1 < NTC:
                    emit_vmix(qs + 1)
                for hp in range(2):
                    emit_attn(hp, qs)

            # ================= phase 3: out = y @ Wo (row-parallel partial) ====
            for os_ in range(4):
                wo_sl = wop.tile([128, 4, 512], BF16, tag="wo")
                nc.sync.dma_start(
                    out=wo_sl,
                    in_=wo.rearrange("(h d) o -> d h o", d=128)[:, :, os_ * 512:(os_ + 1) * 512],
                )
                for tt in range(T // 128):
                    pool3, tag3 = (p_s, "s") if tt % 2 == 0 else (p_or, "or")
                    po = pool3.tile([128, 512], F32, tag=tag3)
                    for h in range(4):
                        nc.tensor.matmul(po, qyT_sb[:, h, tt * 128:(tt + 1) * 128],
                                         wo_sl[:, h, :], start=(h == 0), stop=(h == 3))
                    stage = stg.tile([128, 512], BF16, tag="stage")
                    if tt % 2 == 0:
                        nc.vector.tensor_copy(stage, po)
                    else:
                        nc.scalar.copy(stage, po)
                    nc.sync.dma_start(
                        out=out[tt * 128:(tt + 1) * 128, os_ * 512:(os_ + 1) * 512],
                        in_=stage)

    nc.compile()
    return nc


def _masks():
    jj = np.arange(128)[:, None]
    ii = np.arange(128)[None, :]
    tri_d = (jj <= ii).astype(np.float32)   # diag block: keep j <= i
    tri_f = (jj >= ii).astype(np.float32)   # far block: keep j >= i - WIN
    one = np.ones((128, 128), np.float32)
    zero = np.zeros((128, 128), np.float32)
    m0 = np.concatenate([tri_d, one], 1)
    m1 = np.concatenate([zero, tri_d], 1)
    m2 = np.concatenate([tri_f, zero], 1)
    m3 = np.concatenate([one, tri_f], 1)
    base = np.ascontiguousarray(np.tile(np.stack([m0, m1, m2, m3]), (1, 1, 2)))
    mn = np.ascontiguousarray(np.concatenate([tri_d, tri_d], 1))
    return base, mn


def _hilo(a, scale=1.0):
    import ml_dtypes
    F8 = ml_dtypes.float8_e4m3
    s = (a * scale).astype(np.float32)
    h = s.astype(F8)
    l = (s - h.astype(np.float32)).astype(F8)
    return np.ascontiguousarray(h), np.ascontiguousarray(l)


def _pack_x(a):
    # [E, T] -> chunk-major [NTC, 128, NE, TC] (partition p owns row 128e+p)
    return np.ascontiguousarray(
        a.reshape(NE, 128, NTC, TC).transpose(2, 1, 0, 3))


def _pack_w(a):
    # [E, D] -> partition-major [128, NE, D]
    return np.ascontiguousarray(a.reshape(NE, 128, -1).transpose(1, 0, 2))


def kernel(**inputs):
    import ml_dtypes
    from concourse.bass_utils import run_bass_kernel_spmd

    BF = ml_dtypes.bfloat16

    if "nc" not in _CACHE:
        _CACHE["nc"] = _build_program()
    nc = _CACHE["nc"]

    x = np.asarray(inputs["x"], np.float32)
    ve = np.asarray(inputs["ve"], np.float32)
    cos = np.asarray(inputs["cos"], np.float32)
    sin = np.asarray(inputs["sin"], np.float32)
    Wq = np.asarray(inputs["Wq"], np.float32)
    Wk = np.asarray(inputs["Wk"], np.float32)
    Wv = np.asarray(inputs["Wv"], np.float32)
    Wo = np.asarray(inputs["Wo"], np.float32)
    Wg = np.asarray(inputs["Wg"], np.float32)

    crep = np.ascontiguousarray(np.concatenate([cos.T, cos.T], 0)).astype(BF)
    ssgn = np.ascontiguousarray(np.concatenate([sin.T, -sin.T], 0)).astype(BF)
    masks, masksn = _masks()
    masks = masks.astype(BF)
    masksn = masksn.astype(BF)
    ones128 = np.ones((128, 128), BF)
    eye128 = np.eye(128, dtype=BF)

    in_maps = []
    for c in range(8):
        b, g = divmod(c, 4)
        xT = np.ascontiguousarray(x[b].T)
        xh, xl = _hilo(xT)
        wq_h, wq_l = _hilo(Wq[:, g * 512:(g + 1) * 512], WSC)
        wk_h, wk_l = _hilo(Wk[:, g * HD:(g + 1) * HD], WSC)
        wv_h, wv_l = _hilo(Wv[:, g * HD:(g + 1) * HD], WSC)
        xh, xl = _pack_x(xh), _pack_x(xl)
        wq_h, wq_l = _pack_w(wq_h), _pack_w(wq_l)
        wk_h, wk_l = _pack_w(wk_h), _pack_w(wk_l)
        wv_h, wv_l = _pack_w(wv_h), _pack_w(wv_l)
        in_maps.append({
            "xh8": xh,
            "xl8": xl,
            "xg": np.ascontiguousarray(xT[:GATE_C]).astype(BF),
            "veT": np.ascontiguousarray(ve[b, :, g * HD:(g + 1) * HD].T).astype(BF),
            "crep": crep,
            "ssgn": ssgn,
            "wqh": wq_h, "wql": wq_l,
            "wkh": wk_h, "wkl": wk_l,
            "wvh": wv_h, "wvl": wv_l,
            "wg": np.ascontiguousarray(np.repeat(Wg[:, g:g + 1], 128, 1)).astype(BF),
            "wo": np.ascontiguousarray(Wo[g * 512:(g + 1) * 512, :] / WSC).astype(BF),
            "m_in": masks,
            "mn_in": masksn,
            "ones_in": ones128,
            "eye_in": eye128,
        })

    res = run_bass_kernel_spmd(nc, in_maps, core_ids=list(range(8)))
    parts = [np.asarray(res.results[c]["out"]).astype(np.float32) for c in range(8)]
    out = np.stack([parts[0] + parts[1] + parts[2] + parts[3],
                    parts[4] + parts[5] + parts[6] + parts[7]])
    return out.astype(np.float32)
